# revision 33
# baseline (speedup 1.0000x reference)
"""ConvexSH ColBERT loss kernel for 8 trn2 NeuronCores (v2).

Shards batch B=128 over 8 cores (16 rows each). Each core sees all NWAY=8
candidates for its rows, so softmax + loss are core-local; the host averages
the 8 partial sums.

v2 layout: partition p = (b, c) holds a CONTIGUOUS 32-token chunk c of row b
(16 KiB source runs -> 128 DMA descriptors per candidate instead of 2048).

Per-candidate pipeline (stage offsets in iterations):
  u+0: ACT Square (bf16) of the raw doc block
  u+1: DVE fold1 (bf16 2x), GPSIMD fold2 + reduce -> ssq
  u+2: ACT rsqrt via Ln/Exp (single act table, manually pinned),
       GPSIMD scale2 = mask*rsqrt duplicated into bf16 pairs,
       DVE normalize via pair-broadcast (2x_1P), PE transposes,
       ACT/DVE PSUM evac (ACT uses int32-bitcast copies), PE matmuls
  u+3: DVE reduce_max from f32 PSUM
Last two candidates are split into half-size units to shorten the drain.
Tail: batched softmax + ConvexSH loss on [4,32] tiles, partial sum to host.
"""

import sys
from contextlib import ExitStack

import numpy as np

for _p in ("/opt/trn_rl_repo", "/root/.axon_site/_ro/trn_rl_repo"):
    if _p not in sys.path:
        sys.path.append(_p)

import concourse.bacc as bacc
import concourse.tile as tile
from concourse import mybir
from concourse.bass_utils import run_bass_kernel_spmd

AF = mybir.ActivationFunctionType
AX = mybir.AxisListType
ALU = mybir.AluOpType
F32 = mybir.dt.float32
BF16 = mybir.dt.bfloat16
U32 = mybir.dt.uint32

NCORES = 8
B, LQ, LD, D, NWAY = 128, 32, 256, 128, 8
BS = B // NCORES   # 16 batch rows per core
NG = BS // 4       # 4 groups of 4 rows (PSUM partition packing)
NCH = 8            # token chunks per row; partition p = b*NCH + c
KP = LD // NCH     # 32 tokens per partition per candidate
ALPHA, GAMMA = 0.2, 2.0

TRACE = False
LAST_RESULTS = None

# ---- tuning knobs ----
# NOTE: ACT evac must copy as BF16 (not int32-bitcast): the ACT datapath is
# reduced-precision fp internally and mangles the low 16 bits of u32 words.
EVAC_ENG = "DADA"        # per 8-k' chunk: A=ACT(bf16 copy) D=DVE(bf16 2x)
MANUAL_ACT_TABLE = True  # pin natural_log_exp_and_others (id 6) once
ACT_TABLE_ID = 6
# units per candidate: first/last candidates are split finer to shorten the
# pipeline ramp (first compute starts sooner) and drain (shorter dep chain)
SPLITS = {0: 2, 5: 2, 6: 2, 7: 4}
DEBUG_OUT = False        # dump maxs/p for numeric bisection
NORM_PAIR = True         # pair-broadcast 2x normalize vs safe 1x broadcast


def _build():
    nc = bacc.Bacc("TRN2", target_bir_lowering=False, detect_race_conditions=False)

    q_d = nc.dram_tensor("q", [BS, LQ, D], F32, kind="ExternalInput")
    doc_d = nc.dram_tensor("doc", [NWAY, BS, LD, D], F32, kind="ExternalInput")
    mask_d = nc.dram_tensor("mask", [NWAY, BS, LD], F32, kind="ExternalInput")
    lab_d = nc.dram_tensor("lab", [BS, 3 * NWAY], F32, kind="ExternalInput")
    eye_d = nc.dram_tensor("eye", [128, 128], F32, kind="ExternalInput")
    y_d = nc.dram_tensor("y", [4, 1], F32, kind="ExternalOutput")
    if DEBUG_OUT:
        dbg_maxs_d = nc.dram_tensor("dbg_maxs", [128, NG * NWAY], F32,
                                    kind="ExternalOutput")
        dbg_p_d = nc.dram_tensor("dbg_p", [4, NG * NWAY], F32,
                                 kind="ExternalOutput")
        dbg_sc2_d = nc.dram_tensor("dbg_sc2", [128, KP, 2], BF16,
                                   kind="ExternalOutput")
        dbg_ssq_d = nc.dram_tensor("dbg_ssq", [128, KP], F32,
                                   kind="ExternalOutput")
        dbg_dt_d = nc.dram_tensor("dbg_dt", [128, KP, 128], BF16,
                                  kind="ExternalOutput")
        dbg_qt_d = nc.dram_tensor("dbg_qt", [128, BS * LQ], BF16,
                                  kind="ExternalOutput")

    # work units: (cand, lo, hi) in k' space
    units = []
    for n in range(NWAY):
        ns = SPLITS.get(n, 1)
        step = KP // ns
        for s in range(ns):
            units.append((n, s * step, (s + 1) * step))
    NU = len(units)

    with tile.TileContext(nc) as tc, ExitStack() as ctx:
        singles = ctx.enter_context(tc.tile_pool(name="singles", bufs=1))
        dnpool = ctx.enter_context(tc.tile_pool(name="dnpool", bufs=NU))
        sqpool = ctx.enter_context(tc.tile_pool(name="sqpool", bufs=3))
        fpool = ctx.enter_context(tc.tile_pool(name="fpool", bufs=3))
        spool = ctx.enter_context(tc.tile_pool(name="spool", bufs=3))
        dtpool = ctx.enter_context(tc.tile_pool(name="dtpool", bufs=3))
        psT = ctx.enter_context(tc.tile_pool(name="psT", bufs=3, space="PSUM"))
        psMM = ctx.enter_context(tc.tile_pool(name="psMM", bufs=2, space="PSUM"))
        psS = ctx.enter_context(tc.tile_pool(name="psS", bufs=1, space="PSUM"))

        # pin the one activation table (ln/exp/square/copy all live in set 6)
        if MANUAL_ACT_TABLE:
            nc.scalar.add_instruction(mybir.InstLoadActFuncSet(
                name=nc.get_next_instruction_name(), ins=[], outs=[],
                act_func_set_id=ACT_TABLE_ID))

        # ---- input DMAs ---------------------------------------------------
        # q/eye/mask go on the SWDGE (pool) ring BEFORE the doc blocks so
        # they complete first (HWDGE smalls starve behind the doc stream);
        # labels ride the otherwise-empty HWDGE ring.
        q_nat = singles.tile([128, NG, D], BF16)
        nc.gpsimd.dma_start(out=q_nat,
                            in_=q_d.rearrange("(t r) q d -> (r q) t d", r=4))
        eye_sb = singles.tile([128, 128], BF16)
        nc.gpsimd.dma_start(out=eye_sb, in_=eye_d[:, :])
        lab_sb = singles.tile([4, NG, 3 * NWAY], F32)
        nc.sync.dma_start(out=lab_sb, in_=lab_d.rearrange("(g m) c -> m g c", m=4))

        # ---- doc DMAs: cast f32->bf16 via SWDGE, contiguous 16KiB runs,
        # all generated up-front so the queues never run dry --------------
        dns = {}
        maskA = singles.tile([128, NWAY, KP], BF16)

        def dma_issue(u):
            n, lo, hi = units[u]
            dn = dnpool.tile([128, KP, D], BF16, tag="dn", name=f"dn{u}")
            nc.gpsimd.dma_start(
                out=dn[:, lo:hi, :],
                in_=doc_d[n].rearrange("b (c k) d -> (b c) k d", c=NCH)[:, lo:hi, :])
            dns[u] = dn

        dma_issue(0)
        # masks after the first doc block: partition (b,c), free (n, k');
        # 0/1 so the bf16 cast is exact
        nc.gpsimd.dma_start(out=maskA,
                            in_=mask_d.rearrange("n b (c k) -> (b c) n k", c=NCH))
        for u in range(1, NU):
            dma_issue(u)

        # ---- query prep (overlaps DMA ramp) ------------------------------
        ssq_q = singles.tile([128, NG], F32)
        for t in range(NG):
            sq_t = sqpool.tile([128, KP, D], BF16, tag="sq")
            nc.vector.scalar_tensor_tensor(
                out=sq_t[:, 0, :], in0=q_nat[:, t, :], scalar=1.0,
                in1=q_nat[:, t, :], op0=ALU.mult, op1=ALU.mult,
                accum_out=ssq_q[:, t:t + 1])
        invq = singles.tile([128, NG], F32)
        nc.scalar.activation(out=invq, in_=ssq_q, func=AF.Ln)
        nc.scalar.activation(out=invq, in_=invq, func=AF.Exp, scale=-0.5)
        # qhat = q * invq (folds the query norm into the matmul lhsT)
        nc.vector.tensor_mul(q_nat, q_nat,
                             invq[:, :, None].to_broadcast((128, NG, D)))

        qT = singles.tile([128, BS * LQ], BF16)  # [d, b*32+q]
        psq = psT.tile([128, 4, 128], BF16, tag="psT")
        for t in range(NG):
            nc.tensor.transpose(psq[:, t, :], q_nat[:, t, :], eye_sb)
        nc.scalar.copy(qT, psq.rearrange("p a b -> p (a b)"))

        blockones = singles.tile([128, NG], F32)
        nc.vector.memset(blockones, 0.0)
        for m in range(4):
            nc.vector.memset(blockones[m * 32:(m + 1) * 32, m:m + 1], 1.0)
        # ---- label-only precompute (runs during the DMA ramp) ------------
        t3 = lab_sb[:, :, 0:NWAY]
        r3 = lab_sb[:, :, NWAY:2 * NWAY]
        w3 = lab_sb[:, :, 2 * NWAY:3 * NWAY]

        def t32(name):
            t = singles.tile([4, NG * NWAY], F32, tag=name)
            return t, t.rearrange("p (g n) -> p g n", g=NG)

        a, a3 = t32("a")        # 2w - 1
        b1, b13 = t32("b1")     # 1 - w
        nc.vector.tensor_scalar(out=a3, in0=w3, scalar1=2.0, scalar2=-1.0,
                                op0=ALU.mult, op1=ALU.add)
        nc.vector.tensor_scalar(out=b13, in0=w3, scalar1=-1.0, scalar2=1.0,
                                op0=ALU.mult, op1=ALU.add)
        rr, rr3 = t32("rr")
        nc.vector.reciprocal(rr3, r3)
        wts, wts3 = t32("wts")
        nc.vector.tensor_scalar(out=wts, in0=rr, scalar1=-ALPHA, scalar2=GAMMA,
                                op0=ALU.mult, op1=ALU.add)
        nc.vector.scalar_tensor_tensor(
            out=wts3, in0=rr3[:, :, 0:1].to_broadcast((4, NG, NWAY)),
            scalar=ALPHA, in1=wts3, op0=ALU.mult, op1=ALU.add)

        # maxs[p=(m,q), g*NWAY+n]
        maxs = singles.tile([128, NG * NWAY], F32)
        maxs3 = maxs.rearrange("p (g n) -> p g n", n=NWAY)

        # ---- per-unit state ----------------------------------------------
        sqs, ssqs, invns, sc2s, dts, sims = {}, {}, {}, {}, {}, {}

        def s1_square(u):
            n, lo, hi = units[u]
            dn = dns[u]
            kw = hi - lo
            sq = sqpool.tile([128, KP, D], BF16, tag="sq", name=f"sq{u}")
            nc.scalar.activation(
                out=sq[:, lo:hi, :].rearrange("p k d -> p (k d)"),
                in_=dn[:, lo:hi, :].rearrange("p k d -> p (k d)"),
                func=AF.Square)
            sqs[u] = sq

        def s2_ssq(u):
            # fold1 (DVE bf16 2x, early in the iteration)
            n, lo, hi = units[u]
            sq = sqs[u]
            nc.vector.tensor_add(sq[:, lo:hi, 0:64], sq[:, lo:hi, 0:64],
                                 sq[:, lo:hi, 64:128])

        def s2b_ssq(u):
            # fold2/3/4 (GPSIMD), reduce (DVE), rsqrt (ACT rear), scale (GPSIMD)
            n, lo, hi = units[u]
            sq = sqs.pop(u)
            fold = fpool.tile([128, KP, 32], BF16, tag="fold", name=f"fold{u}")
            nc.gpsimd.tensor_add(fold[:, lo:hi, :], sq[:, lo:hi, 0:32],
                                 sq[:, lo:hi, 32:64])
            nc.gpsimd.tensor_add(fold[:, lo:hi, 0:16], fold[:, lo:hi, 0:16],
                                 fold[:, lo:hi, 16:32])
            nc.gpsimd.tensor_add(fold[:, lo:hi, 0:8], fold[:, lo:hi, 0:8],
                                 fold[:, lo:hi, 8:16])
            ssq = spool.tile([128, KP], F32, tag="ssq", name=f"ssq{u}")
            nc.vector.reduce_sum(out=ssq[:, lo:hi], in_=fold[:, lo:hi, 0:8],
                                 axis=AX.X)
            # rsqrt = exp(-0.5*ln(x)); ssq of randn rows is never near zero
            invn = spool.tile([128, KP], F32, tag="invn", name=f"invn{u}")
            nc.scalar.activation(out=invn[:, lo:hi], in_=ssq[:, lo:hi], func=AF.Ln)
            nc.scalar.activation(out=invn[:, lo:hi], in_=invn[:, lo:hi],
                                 func=AF.Exp, scale=-0.5)
            # scale2[p,k',j] = mask*invn for j=0,1 (pair layout enables 2x mult)
            sc2 = spool.tile([128, KP, 2], BF16, tag="sc2", name=f"sc2{u}")
            for j in range(2):
                nc.gpsimd.tensor_mul(sc2[:, lo:hi, j], maskA[:, n, lo:hi],
                                     invn[:, lo:hi])
            sc2s[u] = sc2
            if DEBUG_OUT and u == 0:
                nc.sync.dma_start(out=dbg_sc2_d[:, :, :], in_=sc2)
                nc.sync.dma_start(out=dbg_ssq_d[:, :], in_=ssq)

        def s3_norm(u):
            # normalize in place, two halves so transposes can start early;
            # bf16 pairs with innermost step 1 on both operands -> 2x_1P
            n, lo, hi = units[u]
            sc2 = sc2s.pop(u)
            dn = dns[u]
            mid = (lo + hi) // 2
            for (l0, h0) in ((lo, mid), (mid, hi)):
                if NORM_PAIR:
                    dnp = dn[:, l0:h0, :].rearrange("p k (e t) -> p k e t", t=2)
                    nc.vector.tensor_mul(
                        dnp, dnp,
                        sc2[:, l0:h0, None, :].to_broadcast(
                            (128, h0 - l0, D // 2, 2)))
                else:
                    nc.vector.tensor_mul(
                        dn[:, l0:h0, :], dn[:, l0:h0, :],
                        sc2[:, l0:h0, 0:1].to_broadcast((128, h0 - l0, D)))

        def s4_transpose_evac(u):
            n, lo, hi = units[u]
            dn = dns.pop(u)
            dt = dtpool.tile([128, KP, 128], BF16, tag="dt", name=f"dt{u}")
            nch = (hi - lo) // 8
            for j in range(nch):
                ps = psT.tile([128, 8, 128], BF16, tag="psT")
                for jj in range(8):
                    kk = lo + j * 8 + jj
                    nc.tensor.transpose(ps[:, jj, :], dn[:, kk, :], eye_sb)
                dst = dt[:, lo + j * 8:lo + j * 8 + 8, :]
                if EVAC_ENG[j % len(EVAC_ENG)] == "A":
                    nc.scalar.copy(dst.rearrange("p a b -> p (a b)"),
                                   ps.rearrange("p a b -> p (a b)"))
                else:
                    nc.vector.tensor_copy(dst, ps)
            dts[u] = dt

        def s5_matmul(u):
            n, lo, hi = units[u]
            dt = dts.pop(u)
            if DEBUG_OUT and u == 0:
                nc.sync.dma_start(out=dbg_dt_d[:, :, :], in_=dt)
                nc.sync.dma_start(out=dbg_qt_d[:, :], in_=qT)
            if n not in sims:
                sims[n] = psMM.tile([128, NG, LD], F32, tag="sim", name=f"sim{n}")
            sim = sims[n]
            for g in range(NG):
                for m in range(4):
                    b = g * 4 + m
                    nc.tensor.matmul(
                        sim[m * 32:(m + 1) * 32, g, lo * NCH:hi * NCH],
                        lhsT=qT[:, b * 32:(b + 1) * 32],
                        rhs=dt[:, lo:hi, b * NCH:(b + 1) * NCH],
                        start=True, stop=True,
                        tile_position=(0, m * 32))

        def s6_maxred(n):
            sim = sims.pop(n)
            nc.vector.reduce_max(out=maxs3[:, :, n], in_=sim, axis=AX.X)

        # last unit index per candidate (for maxred scheduling)
        last_unit = {}
        for u, (n, lo, hi) in enumerate(units):
            last_unit[n] = u

        # ---- software-pipelined main loop --------------------------------
        # iteration k issue order (per-engine queues):
        #   DVE: fold1(k-1), norm halves(k-2), evac-D(k-2), maxred(k-4)
        #   ACT: square(k), evac-A x3(k-2), ln/exp(k-1)
        #   GPS: fold2+reduce(k-1), sc2(k-1), dma-gen(k+3)
        #   PE:  matmuls(k-3), transposes(k-2)
        for k in range(NU + 4):
            if 1 <= k < NU + 1:
                s2_ssq(k - 1)            # DVE fold1
            if 2 <= k < NU + 2:
                s3_norm(k - 2)           # DVE norm halves
            if k < NU:
                s1_square(k)             # ACT
            if 3 <= k < NU + 3:
                s5_matmul(k - 3)         # PE (first in PE queue)
            if 2 <= k < NU + 2:
                s4_transpose_evac(k - 2)  # PE + ACT/DVE evacs
            if 4 <= k:
                for n0, ul in last_unit.items():
                    if ul == k - 4:
                        s6_maxred(n0)    # DVE (last)
            if 1 <= k < NU + 1:
                s2b_ssq(k - 1)           # GPS folds/reduce, ACT rsqrt, GPS sc2

        # ---- scores: per-row sum of maxes via blockones matmul -----------
        if DEBUG_OUT:
            nc.sync.dma_start(out=dbg_maxs_d[:, :], in_=maxs)
        scores_ps = psS.tile([4, NG * NWAY], F32, tag="x")
        nc.tensor.matmul(scores_ps, lhsT=blockones, rhs=maxs, start=True, stop=True)
        sc = singles.tile([4, NG * NWAY], F32)  # [m, g*8+n] = scores[b=g*4+m, n]
        nc.vector.tensor_copy(sc, scores_ps)
        sc3 = sc.rearrange("p (g n) -> p g n", n=NWAY)

        # ---- batched softmax over n (|scores| <= LQ so exp cannot
        # overflow f32; skip the usual max-subtraction) --------------------
        nc.scalar.activation(out=sc, in_=sc, func=AF.Exp)
        sm = singles.tile([4, NG], F32)
        nc.vector.reduce_sum(out=sm, in_=sc3, axis=AX.X)
        nc.vector.reciprocal(sm, sm)
        nc.vector.tensor_mul(sc3, sc3, sm[:, :, None].to_broadcast((4, NG, NWAY)))
        # sc now holds p = softmax(scores)
        if DEBUG_OUT:
            nc.sync.dma_start(out=dbg_p_d[:, :], in_=sc)

        # ---- ConvexSH loss (batched [4, NG*NWAY] ops) --------------------
        # pack [p2 | tinv | omp2] into one tile -> one Ln call
        pk = singles.tile([4, 3, NG * NWAY], F32)
        p2 = pk[:, 0, :]
        tinv = pk[:, 1, :]
        omp2 = pk[:, 2, :]
        nc.vector.tensor_mul(p2, a, sc)
        nc.vector.tensor_add(p2, p2, b1)
        nc.vector.tensor_mul(pk[:, 1, :].rearrange("p (g n) -> p g n", g=NG), a3, t3)
        nc.vector.tensor_add(tinv, tinv, b1)
        nc.vector.tensor_scalar(out=omp2, in0=p2, scalar1=-1.0, scalar2=1.0,
                                op0=ALU.mult, op1=ALU.add)
        lpk = singles.tile([4, 3, NG * NWAY], F32)
        nc.scalar.activation(out=lpk.rearrange("p a b -> p (a b)"),
                             in_=pk.rearrange("p a b -> p (a b)"), func=AF.Ln)
        lp = lpk[:, 0, :]       # ln(p2)
        lt = lpk[:, 1, :]       # ln(t_inv)
        lo_ = lpk[:, 2, :]      # ln(1-p2)

        losses, losses3 = t32("losses")
        nc.vector.tensor_sub(losses, lt, lp)
        nc.vector.tensor_mul(losses3, losses3, t3)

        # pack [wts*ln(1-p2) | wts*ln(p2)] -> one Exp call
        pw = singles.tile([4, 2, NG * NWAY], F32)
        nc.vector.tensor_mul(pw[:, 0, :], lo_, wts)
        nc.vector.tensor_mul(pw[:, 1, :], lp, wts)
        nc.scalar.activation(out=pw.rearrange("p a b -> p (a b)"),
                             in_=pw.rearrange("p a b -> p (a b)"), func=AF.Exp)

        lv, lv3 = t32("lv")
        nc.vector.tensor_mul(lv3, w3, pw[:, 0, :].rearrange("p (g n) -> p g n", g=NG))
        t2, t23 = t32("t2")
        nc.vector.tensor_mul(t23, b13, pw[:, 1, :].rearrange("p (g n) -> p g n", g=NG))
        nc.vector.tensor_add(lv, lv, t2)
        nc.vector.tensor_mul(lv, lv, losses)

        # per-partition partial sums; the host adds the final 4 values
        partial = singles.tile([4, 1], F32)
        nc.vector.reduce_sum(out=partial, in_=lv, axis=AX.X)
        nc.sync.dma_start(out=y_d[:, :], in_=partial)

    nc.finalize()
    return nc


_nc_cache = None


def kernel(query_reps, doc_reps, doc_masks, labels):
    global _nc_cache, LAST_RESULTS
    if _nc_cache is None:
        _nc_cache = _build()
    nc = _nc_cache

    eye = np.eye(128, dtype=np.float32)
    in_maps = []
    for c in range(NCORES):
        sl = slice(c * BS, (c + 1) * BS)
        in_maps.append({
            "q": np.ascontiguousarray(query_reps[sl]).astype(np.float32, copy=False),
            "doc": np.ascontiguousarray(doc_reps[:, sl]).astype(np.float32, copy=False),
            "mask": np.ascontiguousarray(doc_masks[:, sl]).astype(np.float32, copy=False),
            "lab": np.ascontiguousarray(labels[sl]).astype(np.float32, copy=False),
            "eye": eye,
        })

    kwargs = {}
    if TRACE:
        kwargs["trace"] = True
    res = run_bass_kernel_spmd(nc, in_maps, core_ids=list(range(NCORES)), **kwargs)
    LAST_RESULTS = res
    total = sum(float(np.asarray(res.results[c]["y"]).sum()) for c in range(NCORES))
    return np.array(total / (B * NWAY), dtype=np.float32)


# revision 34
# speedup vs baseline: 1.1798x; 1.1798x over previous
"""ConvexSH ColBERT loss kernel for 8 trn2 NeuronCores (v2).

Shards batch B=128 over 8 cores (16 rows each). Each core sees all NWAY=8
candidates for its rows, so softmax + loss are core-local; the host averages
the 8 partial sums.

v2 layout: partition p = (b, c) holds a CONTIGUOUS 32-token chunk c of row b
(16 KiB source runs -> 128 DMA descriptors per candidate instead of 2048).

Per-candidate pipeline (stage offsets in iterations):
  u+0: ACT Square (bf16) of the raw doc block
  u+1: DVE fold1 (bf16 2x), GPSIMD fold2 + reduce -> ssq
  u+2: ACT rsqrt via Ln/Exp (single act table, manually pinned),
       GPSIMD scale2 = mask*rsqrt duplicated into bf16 pairs,
       DVE normalize via pair-broadcast (2x_1P), PE transposes,
       ACT/DVE PSUM evac (ACT uses int32-bitcast copies), PE matmuls
  u+3: DVE reduce_max from f32 PSUM
Last two candidates are split into half-size units to shorten the drain.
Tail: batched softmax + ConvexSH loss on [4,32] tiles, partial sum to host.
"""

import sys
from contextlib import ExitStack

import numpy as np

for _p in ("/opt/trn_rl_repo", "/root/.axon_site/_ro/trn_rl_repo"):
    if _p not in sys.path:
        sys.path.append(_p)

import concourse.bacc as bacc
import concourse.tile as tile
from concourse import mybir
from concourse.bass_utils import run_bass_kernel_spmd

AF = mybir.ActivationFunctionType
AX = mybir.AxisListType
ALU = mybir.AluOpType
F32 = mybir.dt.float32
BF16 = mybir.dt.bfloat16
U32 = mybir.dt.uint32

NCORES = 8
B, LQ, LD, D, NWAY = 128, 32, 256, 128, 8
BS = B // NCORES   # 16 batch rows per core
NG = BS // 4       # 4 groups of 4 rows (PSUM partition packing)
NCH = 8            # token chunks per row; partition p = b*NCH + c
KP = LD // NCH     # 32 tokens per partition per candidate
ALPHA, GAMMA = 0.2, 2.0

TRACE = False
LAST_RESULTS = None

# ---- tuning knobs ----
# NOTE: ACT evac must copy as BF16 (not int32-bitcast): the ACT datapath is
# reduced-precision fp internally and mangles the low 16 bits of u32 words.
EVAC_ENG = "DADA"        # per 8-k' chunk: A=ACT(bf16 copy) D=DVE(bf16 2x)
MANUAL_ACT_TABLE = True  # pin natural_log_exp_and_others (id 6) once
ACT_TABLE_ID = 6
# units per candidate: trailing candidates are split to shorten the drain
# (finer splits raise steady-state overhead more than they save)
SPLITS = {6: 2, 7: 2}
DEBUG_OUT = False        # dump maxs/p for numeric bisection
NORM_PAIR = True         # pair-broadcast 2x normalize vs safe 1x broadcast


def _build():
    nc = bacc.Bacc("TRN2", target_bir_lowering=False, detect_race_conditions=False)

    q_d = nc.dram_tensor("q", [BS, LQ, D], F32, kind="ExternalInput")
    doc_d = nc.dram_tensor("doc", [NWAY, BS, LD, D], F32, kind="ExternalInput")
    mask_d = nc.dram_tensor("mask", [NWAY, BS, LD], F32, kind="ExternalInput")
    lab_d = nc.dram_tensor("lab", [BS, 3 * NWAY], F32, kind="ExternalInput")
    eye_d = nc.dram_tensor("eye", [128, 128], F32, kind="ExternalInput")
    y_d = nc.dram_tensor("y", [4, 1], F32, kind="ExternalOutput")
    if DEBUG_OUT:
        dbg_maxs_d = nc.dram_tensor("dbg_maxs", [128, NG * NWAY], F32,
                                    kind="ExternalOutput")
        dbg_p_d = nc.dram_tensor("dbg_p", [4, NG * NWAY], F32,
                                 kind="ExternalOutput")
        dbg_sc2_d = nc.dram_tensor("dbg_sc2", [128, KP, 2], BF16,
                                   kind="ExternalOutput")
        dbg_ssq_d = nc.dram_tensor("dbg_ssq", [128, KP], F32,
                                   kind="ExternalOutput")
        dbg_dt_d = nc.dram_tensor("dbg_dt", [128, KP, 128], BF16,
                                  kind="ExternalOutput")
        dbg_qt_d = nc.dram_tensor("dbg_qt", [128, BS * LQ], BF16,
                                  kind="ExternalOutput")

    # work units: (cand, lo, hi) in k' space
    units = []
    for n in range(NWAY):
        ns = SPLITS.get(n, 1)
        step = KP // ns
        for s in range(ns):
            units.append((n, s * step, (s + 1) * step))
    NU = len(units)

    with tile.TileContext(nc) as tc, ExitStack() as ctx:
        singles = ctx.enter_context(tc.tile_pool(name="singles", bufs=1))
        dnpool = ctx.enter_context(tc.tile_pool(name="dnpool", bufs=NU))
        sqpool = ctx.enter_context(tc.tile_pool(name="sqpool", bufs=3))
        fpool = ctx.enter_context(tc.tile_pool(name="fpool", bufs=3))
        spool = ctx.enter_context(tc.tile_pool(name="spool", bufs=3))
        dtpool = ctx.enter_context(tc.tile_pool(name="dtpool", bufs=3))
        psT = ctx.enter_context(tc.tile_pool(name="psT", bufs=3, space="PSUM"))
        psMM = ctx.enter_context(tc.tile_pool(name="psMM", bufs=2, space="PSUM"))
        psS = ctx.enter_context(tc.tile_pool(name="psS", bufs=1, space="PSUM"))

        # pin the one activation table (ln/exp/square/copy all live in set 6)
        if MANUAL_ACT_TABLE:
            nc.scalar.add_instruction(mybir.InstLoadActFuncSet(
                name=nc.get_next_instruction_name(), ins=[], outs=[],
                act_func_set_id=ACT_TABLE_ID))

        # ---- input DMAs ---------------------------------------------------
        # q/eye/mask go on the SWDGE (pool) ring BEFORE the doc blocks so
        # they complete first (HWDGE smalls starve behind the doc stream);
        # labels ride the otherwise-empty HWDGE ring.
        q_nat = singles.tile([128, NG, D], BF16)
        nc.gpsimd.dma_start(out=q_nat,
                            in_=q_d.rearrange("(t r) q d -> (r q) t d", r=4))
        eye_sb = singles.tile([128, 128], BF16)
        nc.gpsimd.dma_start(out=eye_sb, in_=eye_d[:, :])
        lab_sb = singles.tile([4, NG, 3 * NWAY], F32)
        nc.sync.dma_start(out=lab_sb, in_=lab_d.rearrange("(g m) c -> m g c", m=4))

        # ---- doc DMAs: cast f32->bf16 via SWDGE, contiguous 16KiB runs,
        # all generated up-front so the queues never run dry --------------
        dns = {}
        maskA = singles.tile([128, NWAY, KP], BF16)

        def dma_issue(u):
            n, lo, hi = units[u]
            dn = dnpool.tile([128, KP, D], BF16, tag="dn", name=f"dn{u}")
            nc.gpsimd.dma_start(
                out=dn[:, lo:hi, :],
                in_=doc_d[n].rearrange("b (c k) d -> (b c) k d", c=NCH)[:, lo:hi, :])
            dns[u] = dn

        dma_issue(0)
        # masks after the first doc block: partition (b,c), free (n, k');
        # 0/1 so the bf16 cast is exact
        nc.gpsimd.dma_start(out=maskA,
                            in_=mask_d.rearrange("n b (c k) -> (b c) n k", c=NCH))
        for u in range(1, NU):
            dma_issue(u)

        # ---- query prep (overlaps DMA ramp) ------------------------------
        ssq_q = singles.tile([128, NG], F32)
        for t in range(NG):
            sq_t = sqpool.tile([128, KP, D], BF16, tag="sq")
            nc.vector.scalar_tensor_tensor(
                out=sq_t[:, 0, :], in0=q_nat[:, t, :], scalar=1.0,
                in1=q_nat[:, t, :], op0=ALU.mult, op1=ALU.mult,
                accum_out=ssq_q[:, t:t + 1])
        invq = singles.tile([128, NG], F32)
        nc.scalar.activation(out=invq, in_=ssq_q, func=AF.Ln)
        nc.scalar.activation(out=invq, in_=invq, func=AF.Exp, scale=-0.5)
        # qhat = q * invq (folds the query norm into the matmul lhsT)
        nc.vector.tensor_mul(q_nat, q_nat,
                             invq[:, :, None].to_broadcast((128, NG, D)))

        qT = singles.tile([128, BS * LQ], BF16)  # [d, b*32+q]
        psq = psT.tile([128, 4, 128], BF16, tag="psT")
        for t in range(NG):
            nc.tensor.transpose(psq[:, t, :], q_nat[:, t, :], eye_sb)
        nc.scalar.copy(qT, psq.rearrange("p a b -> p (a b)"))

        blockones = singles.tile([128, NG], F32)
        nc.vector.memset(blockones, 0.0)
        for m in range(4):
            nc.vector.memset(blockones[m * 32:(m + 1) * 32, m:m + 1], 1.0)
        # ---- label-only precompute (runs during the DMA ramp) ------------
        t3 = lab_sb[:, :, 0:NWAY]
        r3 = lab_sb[:, :, NWAY:2 * NWAY]
        w3 = lab_sb[:, :, 2 * NWAY:3 * NWAY]

        def t32(name):
            t = singles.tile([4, NG * NWAY], F32, tag=name)
            return t, t.rearrange("p (g n) -> p g n", g=NG)

        a, a3 = t32("a")        # 2w - 1
        b1, b13 = t32("b1")     # 1 - w
        nc.vector.tensor_scalar(out=a3, in0=w3, scalar1=2.0, scalar2=-1.0,
                                op0=ALU.mult, op1=ALU.add)
        nc.vector.tensor_scalar(out=b13, in0=w3, scalar1=-1.0, scalar2=1.0,
                                op0=ALU.mult, op1=ALU.add)
        rr, rr3 = t32("rr")
        nc.vector.reciprocal(rr3, r3)
        wts, wts3 = t32("wts")
        nc.vector.tensor_scalar(out=wts, in0=rr, scalar1=-ALPHA, scalar2=GAMMA,
                                op0=ALU.mult, op1=ALU.add)
        nc.vector.scalar_tensor_tensor(
            out=wts3, in0=rr3[:, :, 0:1].to_broadcast((4, NG, NWAY)),
            scalar=ALPHA, in1=wts3, op0=ALU.mult, op1=ALU.add)

        # maxs[p=(m,q), g*NWAY+n]
        maxs = singles.tile([128, NG * NWAY], F32)
        maxs3 = maxs.rearrange("p (g n) -> p g n", n=NWAY)

        # ---- per-unit state ----------------------------------------------
        sqs, ssqs, invns, sc2s, dts, sims = {}, {}, {}, {}, {}, {}

        def s1_square(u):
            n, lo, hi = units[u]
            dn = dns[u]
            kw = hi - lo
            sq = sqpool.tile([128, KP, D], BF16, tag="sq", name=f"sq{u}")
            nc.scalar.activation(
                out=sq[:, lo:hi, :].rearrange("p k d -> p (k d)"),
                in_=dn[:, lo:hi, :].rearrange("p k d -> p (k d)"),
                func=AF.Square)
            sqs[u] = sq

        def s2_ssq(u):
            # fold1 (DVE bf16 2x, early in the iteration)
            n, lo, hi = units[u]
            sq = sqs[u]
            nc.vector.tensor_add(sq[:, lo:hi, 0:64], sq[:, lo:hi, 0:64],
                                 sq[:, lo:hi, 64:128])

        def s2b_ssq(u):
            # fold2/3/4 (GPSIMD), reduce (DVE), rsqrt (ACT rear), scale (GPSIMD)
            n, lo, hi = units[u]
            sq = sqs.pop(u)
            fold = fpool.tile([128, KP, 32], BF16, tag="fold", name=f"fold{u}")
            nc.gpsimd.tensor_add(fold[:, lo:hi, :], sq[:, lo:hi, 0:32],
                                 sq[:, lo:hi, 32:64])
            nc.gpsimd.tensor_add(fold[:, lo:hi, 0:16], fold[:, lo:hi, 0:16],
                                 fold[:, lo:hi, 16:32])
            nc.gpsimd.tensor_add(fold[:, lo:hi, 0:8], fold[:, lo:hi, 0:8],
                                 fold[:, lo:hi, 8:16])
            ssq = spool.tile([128, KP], F32, tag="ssq", name=f"ssq{u}")
            nc.vector.reduce_sum(out=ssq[:, lo:hi], in_=fold[:, lo:hi, 0:8],
                                 axis=AX.X)
            # rsqrt = exp(-0.5*ln(x)); ssq of randn rows is never near zero
            invn = spool.tile([128, KP], F32, tag="invn", name=f"invn{u}")
            nc.scalar.activation(out=invn[:, lo:hi], in_=ssq[:, lo:hi], func=AF.Ln)
            nc.scalar.activation(out=invn[:, lo:hi], in_=invn[:, lo:hi],
                                 func=AF.Exp, scale=-0.5)
            # scale2[p,k',j] = mask*invn for j=0,1 (pair layout enables 2x mult)
            sc2 = spool.tile([128, KP, 2], BF16, tag="sc2", name=f"sc2{u}")
            for j in range(2):
                nc.gpsimd.tensor_mul(sc2[:, lo:hi, j], maskA[:, n, lo:hi],
                                     invn[:, lo:hi])
            sc2s[u] = sc2
            if DEBUG_OUT and u == 0:
                nc.sync.dma_start(out=dbg_sc2_d[:, :, :], in_=sc2)
                nc.sync.dma_start(out=dbg_ssq_d[:, :], in_=ssq)

        def s3_norm(u):
            # normalize in place, two halves so transposes can start early;
            # bf16 pairs with innermost step 1 on both operands -> 2x_1P
            n, lo, hi = units[u]
            sc2 = sc2s.pop(u)
            dn = dns[u]
            mid = (lo + hi) // 2
            for (l0, h0) in ((lo, mid), (mid, hi)):
                if NORM_PAIR:
                    dnp = dn[:, l0:h0, :].rearrange("p k (e t) -> p k e t", t=2)
                    nc.vector.tensor_mul(
                        dnp, dnp,
                        sc2[:, l0:h0, None, :].to_broadcast(
                            (128, h0 - l0, D // 2, 2)))
                else:
                    nc.vector.tensor_mul(
                        dn[:, l0:h0, :], dn[:, l0:h0, :],
                        sc2[:, l0:h0, 0:1].to_broadcast((128, h0 - l0, D)))

        def s4_transpose_evac(u):
            n, lo, hi = units[u]
            dn = dns.pop(u)
            dt = dtpool.tile([128, KP, 128], BF16, tag="dt", name=f"dt{u}")
            nch = (hi - lo) // 8
            for j in range(nch):
                ps = psT.tile([128, 8, 128], BF16, tag="psT")
                for jj in range(8):
                    kk = lo + j * 8 + jj
                    nc.tensor.transpose(ps[:, jj, :], dn[:, kk, :], eye_sb)
                dst = dt[:, lo + j * 8:lo + j * 8 + 8, :]
                if EVAC_ENG[j % len(EVAC_ENG)] == "A":
                    nc.scalar.copy(dst.rearrange("p a b -> p (a b)"),
                                   ps.rearrange("p a b -> p (a b)"))
                else:
                    nc.vector.tensor_copy(dst, ps)
            dts[u] = dt

        def s5_matmul(u):
            n, lo, hi = units[u]
            dt = dts.pop(u)
            if DEBUG_OUT and u == 0:
                nc.sync.dma_start(out=dbg_dt_d[:, :, :], in_=dt)
                nc.sync.dma_start(out=dbg_qt_d[:, :], in_=qT)
            if n not in sims:
                sims[n] = psMM.tile([128, NG, LD], F32, tag="sim", name=f"sim{n}")
            sim = sims[n]
            for g in range(NG):
                for m in range(4):
                    b = g * 4 + m
                    nc.tensor.matmul(
                        sim[m * 32:(m + 1) * 32, g, lo * NCH:hi * NCH],
                        lhsT=qT[:, b * 32:(b + 1) * 32],
                        rhs=dt[:, lo:hi, b * NCH:(b + 1) * NCH],
                        start=True, stop=True,
                        tile_position=(0, m * 32))

        def s6_maxred(n):
            sim = sims.pop(n)
            nc.vector.reduce_max(out=maxs3[:, :, n], in_=sim, axis=AX.X)

        # last unit index per candidate (for maxred scheduling)
        last_unit = {}
        for u, (n, lo, hi) in enumerate(units):
            last_unit[n] = u

        # ---- software-pipelined main loop --------------------------------
        # iteration k issue order (per-engine queues):
        #   DVE: fold1(k-1), norm halves(k-2), evac-D(k-2), maxred(k-4)
        #   ACT: square(k), evac-A x3(k-2), ln/exp(k-1)
        #   GPS: fold2+reduce(k-1), sc2(k-1), dma-gen(k+3)
        #   PE:  matmuls(k-3), transposes(k-2)
        for k in range(NU + 4):
            if 1 <= k < NU + 1:
                s2_ssq(k - 1)            # DVE fold1
            if 2 <= k < NU + 2:
                s3_norm(k - 2)           # DVE norm halves
            if k < NU:
                s1_square(k)             # ACT
            if 3 <= k < NU + 3:
                s5_matmul(k - 3)         # PE (first in PE queue)
            if 2 <= k < NU + 2:
                s4_transpose_evac(k - 2)  # PE + ACT/DVE evacs
            if 4 <= k:
                for n0, ul in last_unit.items():
                    if ul == k - 4:
                        s6_maxred(n0)    # DVE (last)
            if 1 <= k < NU + 1:
                s2b_ssq(k - 1)           # GPS folds/reduce, ACT rsqrt, GPS sc2

        # ---- scores: per-row sum of maxes via blockones matmul -----------
        if DEBUG_OUT:
            nc.sync.dma_start(out=dbg_maxs_d[:, :], in_=maxs)
        scores_ps = psS.tile([4, NG * NWAY], F32, tag="x")
        nc.tensor.matmul(scores_ps, lhsT=blockones, rhs=maxs, start=True, stop=True)
        sc = singles.tile([4, NG * NWAY], F32)  # [m, g*8+n] = scores[b=g*4+m, n]
        nc.vector.tensor_copy(sc, scores_ps)
        sc3 = sc.rearrange("p (g n) -> p g n", n=NWAY)

        # ---- batched softmax over n (|scores| <= LQ so exp cannot
        # overflow f32; skip the usual max-subtraction) --------------------
        nc.scalar.activation(out=sc, in_=sc, func=AF.Exp)
        sm = singles.tile([4, NG], F32)
        nc.vector.reduce_sum(out=sm, in_=sc3, axis=AX.X)
        nc.vector.reciprocal(sm, sm)
        nc.vector.tensor_mul(sc3, sc3, sm[:, :, None].to_broadcast((4, NG, NWAY)))
        # sc now holds p = softmax(scores)
        if DEBUG_OUT:
            nc.sync.dma_start(out=dbg_p_d[:, :], in_=sc)

        # ---- ConvexSH loss (batched [4, NG*NWAY] ops) --------------------
        # pack [p2 | tinv | omp2] into one tile -> one Ln call
        pk = singles.tile([4, 3, NG * NWAY], F32)
        p2 = pk[:, 0, :]
        tinv = pk[:, 1, :]
        omp2 = pk[:, 2, :]
        nc.vector.tensor_mul(p2, a, sc)
        nc.vector.tensor_add(p2, p2, b1)
        nc.vector.tensor_mul(pk[:, 1, :].rearrange("p (g n) -> p g n", g=NG), a3, t3)
        nc.vector.tensor_add(tinv, tinv, b1)
        nc.vector.tensor_scalar(out=omp2, in0=p2, scalar1=-1.0, scalar2=1.0,
                                op0=ALU.mult, op1=ALU.add)
        lpk = singles.tile([4, 3, NG * NWAY], F32)
        nc.scalar.activation(out=lpk.rearrange("p a b -> p (a b)"),
                             in_=pk.rearrange("p a b -> p (a b)"), func=AF.Ln)
        lp = lpk[:, 0, :]       # ln(p2)
        lt = lpk[:, 1, :]       # ln(t_inv)
        lo_ = lpk[:, 2, :]      # ln(1-p2)

        losses, losses3 = t32("losses")
        nc.vector.tensor_sub(losses, lt, lp)
        nc.vector.tensor_mul(losses3, losses3, t3)

        # pack [wts*ln(1-p2) | wts*ln(p2)] -> one Exp call
        pw = singles.tile([4, 2, NG * NWAY], F32)
        nc.vector.tensor_mul(pw[:, 0, :], lo_, wts)
        nc.vector.tensor_mul(pw[:, 1, :], lp, wts)
        nc.scalar.activation(out=pw.rearrange("p a b -> p (a b)"),
                             in_=pw.rearrange("p a b -> p (a b)"), func=AF.Exp)

        lv, lv3 = t32("lv")
        nc.vector.tensor_mul(lv3, w3, pw[:, 0, :].rearrange("p (g n) -> p g n", g=NG))
        t2, t23 = t32("t2")
        nc.vector.tensor_mul(t23, b13, pw[:, 1, :].rearrange("p (g n) -> p g n", g=NG))
        nc.vector.tensor_add(lv, lv, t2)
        nc.vector.tensor_mul(lv, lv, losses)

        # per-partition partial sums; the host adds the final 4 values
        partial = singles.tile([4, 1], F32)
        nc.vector.reduce_sum(out=partial, in_=lv, axis=AX.X)
        nc.sync.dma_start(out=y_d[:, :], in_=partial)

    nc.finalize()
    return nc


_nc_cache = None


def kernel(query_reps, doc_reps, doc_masks, labels):
    global _nc_cache, LAST_RESULTS
    if _nc_cache is None:
        _nc_cache = _build()
    nc = _nc_cache

    eye = np.eye(128, dtype=np.float32)
    in_maps = []
    for c in range(NCORES):
        sl = slice(c * BS, (c + 1) * BS)
        in_maps.append({
            "q": np.ascontiguousarray(query_reps[sl]).astype(np.float32, copy=False),
            "doc": np.ascontiguousarray(doc_reps[:, sl]).astype(np.float32, copy=False),
            "mask": np.ascontiguousarray(doc_masks[:, sl]).astype(np.float32, copy=False),
            "lab": np.ascontiguousarray(labels[sl]).astype(np.float32, copy=False),
            "eye": eye,
        })

    kwargs = {}
    if TRACE:
        kwargs["trace"] = True
    res = run_bass_kernel_spmd(nc, in_maps, core_ids=list(range(NCORES)), **kwargs)
    LAST_RESULTS = res
    total = sum(float(np.asarray(res.results[c]["y"]).sum()) for c in range(NCORES))
    return np.array(total / (B * NWAY), dtype=np.float32)
